# revision 6
# baseline (speedup 1.0000x reference)
"""Density-aware Chamfer distance on 8 Trainium2 NeuronCores.

Problem: pred_points [16384,3], gt_points [16384,3], w_pred/w_gt [16384].
  d2[p,g] = max(|p|^2 + |g|^2 - 2 p.g, 0)
  out = sum(w_pred*min_g d2)/sum(w_pred) + sum(w_gt*min_p d2)/sum(w_gt)

Sharding: pred rows are split across the 8 cores (2048 each). Each core
computes its 2048 x 16384 distance tile entirely on-chip:

 - The d2 matrix block is produced on the TensorEngine as a K=30 bf16
   matmul: d2 = sum_k A[k,g] * B[k,p] with A = [g2, 1, gx, gy, gz] and
   B = [1, p2, -2px, -2py, -2pz], where every product is expanded into
   6 bf16-pair partial products (3-way bf16 split of each fp32 value),
   giving fp32-grade accuracy at full bf16 PE speed (K stays under 128
   so the extra rows are free).
 - Orientation: gt on partitions (128 gt-blocks), pred on the free dim
   (2048). Per block, PSUM holds 1024*d2 [128gt, 2048pred] in fp32 (the
   2^10 scale keeps nearest-neighbour distances in fp16 normal range).
 - ScalarE copies PSUM -> SBUF fp16. VectorE then (a) min-accumulates
   block pairs into a running colacc [128, 2048] (fp16 tensor_tensor at
   2x rate) for the min over gt, and (b) does a pairwise-min tree over
   the free dim (fp16 2x) + an 8-block-grouped reduce for the min over
   pred, which yields each gt-block's min_gt entries (one per lane).
 - Host combines: min_gt = elementwise min over the 8 cores' [128,128]
   block-min outputs; min_pred shard = column-min over the [128,2048]
   colacc; un-scale, clamp at 0 (max(.,0) commutes with min) and the
   weighted means are computed on host in float64.

The max(..., 0) clamp is applied after the min reductions (max(.,0) is
monotone, so it commutes with min).
"""

import numpy as np
import ml_dtypes

import concourse.bacc as bacc
import concourse.tile as tile
import concourse.mybir as mybir
from concourse.bass_utils import run_bass_kernel_spmd

F32 = mybir.dt.float32
F16 = mybir.dt.float16
BF16 = mybir.dt.bfloat16

P = 16384          # pred points
G = 16384          # gt points
NCORES = 8
PSH = P // NCORES  # 2048 pred per core
GB = G // 128      # 128 gt blocks per core
NCH = PSH // 512   # 4 matmul column chunks per block
K = 30             # 5 terms x 6 bf16-pair partial products

PRED_WEIGHT = 1.0
GT_WEIGHT = 1.0
EPS = 1e-9

# bf16-pair partial products kept from (x1+x2+x3)*(y1+y2+y3); dropped
# terms are O(2^-32) relative.
PAIRS = [(0, 0), (0, 1), (1, 0), (1, 1), (0, 2), (2, 0)]

# The on-device min pipeline runs in fp16; d2 is scaled by 2^10 (folded
# into the gt-side matmul rows) so typical nearest-neighbour distances
# (~1e-5) land in fp16's normal range. Overflowed large distances become
# inf, which min() ignores.
SCALE = 1024.0

_CACHED = {}


def _split3(x):
    """3-way bf16 split of a float64 array: x ~= s[0]+s[1]+s[2]."""
    out = []
    r = x
    for _ in range(3):
        h = r.astype(ml_dtypes.bfloat16).astype(np.float64)
        out.append(h)
        r = r - h
    return out


def _expand_rows(A, B):
    """A [5, n], B [5, m] float64 -> (L [30, n], R [30, m]) bf16 with
    sum_k L[k,i]*R[k,j] ~= sum_t A[t,i]*B[t,j]."""
    SA = [_split3(A[t]) for t in range(A.shape[0])]
    SB = [_split3(B[t]) for t in range(B.shape[0])]
    L, R = [], []
    for t in range(A.shape[0]):
        for (i, j) in PAIRS:
            L.append(SA[t][i])
            R.append(SB[t][j])
    return (np.stack(L).astype(ml_dtypes.bfloat16),
            np.stack(R).astype(ml_dtypes.bfloat16))


def _build_device_kernel():
    nc = bacc.Bacc("TRN2", target_bir_lowering=False)
    lg_d = nc.dram_tensor("lg", [K, G], BF16, kind="ExternalInput")
    rp_d = nc.dram_tensor("rp", [K, PSH], BF16, kind="ExternalInput")
    gmin_d = nc.dram_tensor("gmin", [128, GB], F32, kind="ExternalOutput")
    colacc_d = nc.dram_tensor("colacc", [128, PSH], F16, kind="ExternalOutput")

    with tile.TileContext(nc) as tc:
        with (
            tc.tile_pool(name="inp", bufs=1) as inp,
            tc.tile_pool(name="cpp", bufs=3) as cpp,
            tc.tile_pool(name="trp", bufs=2) as trp,
            tc.tile_pool(name="t3p", bufs=2) as t3p,
            tc.tile_pool(name="outp", bufs=1) as outp,
            tc.tile_pool(name="ps", bufs=2, space="PSUM") as ps,
        ):
            lg = inp.tile([K, G], BF16)
            rp = inp.tile([K, PSH], BF16)
            # chunked prefetch so block 0's matmuls start early
            for ch in range(8):
                nc.sync.dma_start(
                    lg[:, ch * (G // 8) : (ch + 1) * (G // 8)],
                    lg_d[:, ch * (G // 8) : (ch + 1) * (G // 8)],
                )
            nc.sync.dma_start(rp[:], rp_d[:])

            colacc = outp.tile([128, PSH], F16)
            nc.vector.memset(colacc[:], 60000.0)
            gmin = outp.tile([128, GB], F32)

            MIN = mybir.AluOpType.min
            # process gt blocks in pairs: one [128, 2*PSH] PSUM batch each
            for grp in range(GB // 2):
                cp = cpp.tile([128, 2 * PSH], F16, tag="cp")
                for half in range(2):
                    gb = 2 * grp + half
                    acc = ps.tile([128, PSH], F32, tag="acc")
                    w = lg[:, 128 * gb : 128 * (gb + 1)]
                    for c in range(NCH):
                        nc.tensor.matmul(
                            acc[:, 512 * c : 512 * (c + 1)],
                            w,
                            rp[:, 512 * c : 512 * (c + 1)],
                            start=True,
                            stop=True,
                        )
                    nc.scalar.copy(cp[:, half * PSH : (half + 1) * PSH], acc[:])

                # colacc via pair-min of the two blocks (min_pred side)
                u = trp.tile([128, PSH], F16, tag="u")
                nc.vector.tensor_tensor(
                    out=u[:], in0=cp[:, :PSH], in1=cp[:, PSH:], op=MIN
                )
                nc.vector.tensor_tensor(
                    out=colacc[:], in0=colacc[:], in1=u[:], op=MIN
                )

                # per-block pairwise-min tree over pred (min_gt side),
                # both blocks folded in the same instruction
                cpr = cp[:].rearrange("p (b h) -> p b h", b=2)
                t1 = trp.tile([128, 2, PSH // 2], F16, tag="t1")
                nc.vector.tensor_tensor(
                    out=t1[:],
                    in0=cpr[:, :, : PSH // 2], in1=cpr[:, :, PSH // 2 :],
                    op=MIN,
                )
                t2 = trp.tile([128, 2, PSH // 4], F16, tag="t2")
                nc.vector.tensor_tensor(
                    out=t2[:],
                    in0=t1[:, :, : PSH // 4], in1=t1[:, :, PSH // 4 :],
                    op=MIN,
                )
                # t3 for this pair goes into the 8-block gather buffer
                j = grp % 4
                if j == 0:
                    t3g = t3p.tile([128, 8, PSH // 8], F16, tag="t3g")
                nc.vector.tensor_tensor(
                    out=t3g[:, 2 * j : 2 * j + 2, :],
                    in0=t2[:, :, : PSH // 8], in1=t2[:, :, PSH // 8 :],
                    op=MIN,
                )
                if j == 3:
                    gb0 = 2 * grp - 6
                    nc.vector.tensor_reduce(
                        gmin[:, gb0 : gb0 + 8], t3g[:],
                        axis=mybir.AxisListType.X, op=MIN,
                    )

            nc.sync.dma_start(gmin_d[:], gmin[:])
            nc.sync.dma_start(colacc_d[:], colacc[:])

    nc.compile()
    return nc


def _get_nc():
    if "nc" not in _CACHED:
        _CACHED["nc"] = _build_device_kernel()
    return _CACHED["nc"]


def kernel(pred_points, gt_points, w_pred, w_gt, _trace=False):
    pred = np.asarray(pred_points, np.float64)
    gt = np.asarray(gt_points, np.float64)
    p2 = (pred * pred).sum(1)
    g2 = (gt * gt).sum(1)

    A = SCALE * np.stack([g2, np.ones(G), gt[:, 0], gt[:, 1], gt[:, 2]])  # [5, G]
    B = np.stack([np.ones(P), p2, -2 * pred[:, 0], -2 * pred[:, 1],
                  -2 * pred[:, 2]])                                     # [5, P]
    Lg, Rp = _expand_rows(A, B)  # [30, G], [30, P] bf16

    nc = _get_nc()
    in_maps = [
        {"lg": Lg, "rp": np.ascontiguousarray(Rp[:, c * PSH : (c + 1) * PSH])}
        for c in range(NCORES)
    ]
    res = None
    for attempt in range(3):
        try:
            res = run_bass_kernel_spmd(
                nc, in_maps, core_ids=list(range(NCORES)), trace=_trace
            )
            break
        except Exception:
            if attempt == 2:
                raise
            import time
            time.sleep(2.0)

    min_gt = np.full(G, np.inf)
    min_pred = np.empty(P)
    for c, out in enumerate(res.results):
        gm = out["gmin"].astype(np.float64)          # [128 lane, GB block]
        min_gt = np.minimum(min_gt, gm.T.reshape(G) / SCALE)  # g = gb*128 + lane
        min_pred[c * PSH : (c + 1) * PSH] = (
            out["colacc"].astype(np.float64).min(axis=0) / SCALE
        )

    min_pred = np.maximum(min_pred, 0.0)
    min_gt = np.maximum(min_gt, 0.0)

    wp = np.asarray(w_pred, np.float64)
    wg = np.asarray(w_gt, np.float64)
    weighted_pred = (wp * min_pred).sum() / max(wp.sum(), EPS)
    weighted_gt = (wg * min_gt).sum() / max(wg.sum(), EPS)
    out = PRED_WEIGHT * weighted_pred + GT_WEIGHT * weighted_gt
    if _trace:
        return np.float32(out), res
    return np.float32(out)


# revision 7
# speedup vs baseline: 1.0002x; 1.0002x over previous
"""Density-aware Chamfer distance on 8 Trainium2 NeuronCores.

Problem: pred_points [16384,3], gt_points [16384,3], w_pred/w_gt [16384].
  d2[p,g] = max(|p|^2 + |g|^2 - 2 p.g, 0)
  out = sum(w_pred*min_g d2)/sum(w_pred) + sum(w_gt*min_p d2)/sum(w_gt)

Sharding: pred rows are split across the 8 cores (2048 each). Each core
computes its 2048 x 16384 distance tile entirely on-chip:

 - The d2 matrix block is produced on the TensorEngine as a K=30 bf16
   matmul: d2 = sum_k A[k,g] * B[k,p] with A = [g2, 1, gx, gy, gz] and
   B = [1, p2, -2px, -2py, -2pz], where every product is expanded into
   6 bf16-pair partial products (3-way bf16 split of each fp32 value),
   giving fp32-grade accuracy at full bf16 PE speed (K stays under 128
   so the extra rows are free).
 - Orientation: gt on partitions (128 gt-blocks), pred on the free dim
   (2048). Per block, PSUM holds 1024*d2 [128gt, 2048pred] in fp32 (the
   2^10 scale keeps nearest-neighbour distances in fp16 normal range).
 - ScalarE copies PSUM -> SBUF fp16. VectorE then (a) min-accumulates
   block pairs into a running colacc [128, 2048] (fp16 tensor_tensor at
   2x rate) for the min over gt, and (b) does a pairwise-min tree over
   the free dim (fp16 2x) + an 8-block-grouped reduce for the min over
   pred, which yields each gt-block's min_gt entries (one per lane).
 - Host combines: min_gt = elementwise min over the 8 cores' [128,128]
   block-min outputs; min_pred shard = column-min over the [128,2048]
   colacc; un-scale, clamp at 0 (max(.,0) commutes with min) and the
   weighted means are computed on host in float64.

The max(..., 0) clamp is applied after the min reductions (max(.,0) is
monotone, so it commutes with min).
"""

import numpy as np
import ml_dtypes

import concourse.bacc as bacc
import concourse.tile as tile
import concourse.mybir as mybir
from concourse.bass_utils import run_bass_kernel_spmd

F32 = mybir.dt.float32
F16 = mybir.dt.float16
BF16 = mybir.dt.bfloat16

P = 16384          # pred points
G = 16384          # gt points
NCORES = 8
PSH = P // NCORES  # 2048 pred per core
GB = G // 128      # 128 gt blocks per core
NCH = PSH // 512   # 4 matmul column chunks per block
K = 30             # 5 terms x 6 bf16-pair partial products

PRED_WEIGHT = 1.0
GT_WEIGHT = 1.0
EPS = 1e-9

# bf16-pair partial products kept from (x1+x2+x3)*(y1+y2+y3); dropped
# terms are O(2^-32) relative.
PAIRS = [(0, 0), (0, 1), (1, 0), (1, 1), (0, 2), (2, 0)]

# The on-device min pipeline runs in fp16; d2 is scaled by 2^10 (folded
# into the gt-side matmul rows) so typical nearest-neighbour distances
# (~1e-5) land in fp16's normal range. Overflowed large distances become
# inf, which min() ignores.
SCALE = 1024.0

_CACHED = {}


def _split3(x):
    """3-way bf16 split of a float64 array: x ~= s[0]+s[1]+s[2]."""
    out = []
    r = x
    for _ in range(3):
        h = r.astype(ml_dtypes.bfloat16).astype(np.float64)
        out.append(h)
        r = r - h
    return out


def _expand_rows(A, B):
    """A [5, n], B [5, m] float64 -> (L [30, n], R [30, m]) bf16 with
    sum_k L[k,i]*R[k,j] ~= sum_t A[t,i]*B[t,j]."""
    SA = [_split3(A[t]) for t in range(A.shape[0])]
    SB = [_split3(B[t]) for t in range(B.shape[0])]
    L, R = [], []
    for t in range(A.shape[0]):
        for (i, j) in PAIRS:
            L.append(SA[t][i])
            R.append(SB[t][j])
    return (np.stack(L).astype(ml_dtypes.bfloat16),
            np.stack(R).astype(ml_dtypes.bfloat16))


def _build_device_kernel():
    nc = bacc.Bacc("TRN2", target_bir_lowering=False)
    lg_d = nc.dram_tensor("lg", [K, G], BF16, kind="ExternalInput")
    rp_d = nc.dram_tensor("rp", [K, PSH], BF16, kind="ExternalInput")
    gmin_d = nc.dram_tensor("gmin", [128, GB], F32, kind="ExternalOutput")
    colacc_d = nc.dram_tensor("colacc", [128, PSH], F16, kind="ExternalOutput")

    with tile.TileContext(nc) as tc:
        with (
            tc.tile_pool(name="inp", bufs=1) as inp,
            tc.tile_pool(name="cpp", bufs=4) as cpp,
            tc.tile_pool(name="trp", bufs=3) as trp,
            tc.tile_pool(name="t3p", bufs=2) as t3p,
            tc.tile_pool(name="outp", bufs=1) as outp,
            tc.tile_pool(name="ps", bufs=2, space="PSUM") as ps,
        ):
            lg = inp.tile([K, G], BF16)
            rp = inp.tile([K, PSH], BF16)
            # chunked prefetch so block 0's matmuls start early
            for ch in range(8):
                nc.sync.dma_start(
                    lg[:, ch * (G // 8) : (ch + 1) * (G // 8)],
                    lg_d[:, ch * (G // 8) : (ch + 1) * (G // 8)],
                )
            nc.sync.dma_start(rp[:], rp_d[:])

            colacc = outp.tile([128, PSH], F16)
            nc.vector.memset(colacc[:], 60000.0)
            gmin = outp.tile([128, GB], F32)

            MIN = mybir.AluOpType.min
            # process gt blocks in pairs: one [128, 2*PSH] PSUM batch each
            for grp in range(GB // 2):
                cp = cpp.tile([128, 2 * PSH], F16, tag="cp")
                for half in range(2):
                    gb = 2 * grp + half
                    acc = ps.tile([128, PSH], F32, tag="acc")
                    w = lg[:, 128 * gb : 128 * (gb + 1)]
                    for c in range(NCH):
                        nc.tensor.matmul(
                            acc[:, 512 * c : 512 * (c + 1)],
                            w,
                            rp[:, 512 * c : 512 * (c + 1)],
                            start=True,
                            stop=True,
                        )
                    nc.scalar.copy(cp[:, half * PSH : (half + 1) * PSH], acc[:])

                # colacc via pair-min of the two blocks (min_pred side)
                u = trp.tile([128, PSH], F16, tag="u")
                nc.vector.tensor_tensor(
                    out=u[:], in0=cp[:, :PSH], in1=cp[:, PSH:], op=MIN
                )
                nc.vector.tensor_tensor(
                    out=colacc[:], in0=colacc[:], in1=u[:], op=MIN
                )

                # per-block pairwise-min tree over pred (min_gt side),
                # both blocks folded in the same instruction
                cpr = cp[:].rearrange("p (b h) -> p b h", b=2)
                t1 = trp.tile([128, 2, PSH // 2], F16, tag="t1")
                nc.vector.tensor_tensor(
                    out=t1[:],
                    in0=cpr[:, :, : PSH // 2], in1=cpr[:, :, PSH // 2 :],
                    op=MIN,
                )
                t2 = trp.tile([128, 2, PSH // 4], F16, tag="t2")
                nc.vector.tensor_tensor(
                    out=t2[:],
                    in0=t1[:, :, : PSH // 4], in1=t1[:, :, PSH // 4 :],
                    op=MIN,
                )
                # t3 for this pair goes into the 8-block gather buffer
                j = grp % 4
                if j == 0:
                    t3g = t3p.tile([128, 8, PSH // 8], F16, tag="t3g")
                nc.vector.tensor_tensor(
                    out=t3g[:, 2 * j : 2 * j + 2, :],
                    in0=t2[:, :, : PSH // 8], in1=t2[:, :, PSH // 8 :],
                    op=MIN,
                )
                if j == 3:
                    gb0 = 2 * grp - 6
                    nc.vector.tensor_reduce(
                        gmin[:, gb0 : gb0 + 8], t3g[:],
                        axis=mybir.AxisListType.X, op=MIN,
                    )

            nc.sync.dma_start(gmin_d[:], gmin[:])
            nc.sync.dma_start(colacc_d[:], colacc[:])

    nc.compile()
    return nc


def _get_nc():
    if "nc" not in _CACHED:
        _CACHED["nc"] = _build_device_kernel()
    return _CACHED["nc"]


def kernel(pred_points, gt_points, w_pred, w_gt, _trace=False):
    pred = np.asarray(pred_points, np.float64)
    gt = np.asarray(gt_points, np.float64)
    p2 = (pred * pred).sum(1)
    g2 = (gt * gt).sum(1)

    A = SCALE * np.stack([g2, np.ones(G), gt[:, 0], gt[:, 1], gt[:, 2]])  # [5, G]
    B = np.stack([np.ones(P), p2, -2 * pred[:, 0], -2 * pred[:, 1],
                  -2 * pred[:, 2]])                                     # [5, P]
    Lg, Rp = _expand_rows(A, B)  # [30, G], [30, P] bf16

    nc = _get_nc()
    in_maps = [
        {"lg": Lg, "rp": np.ascontiguousarray(Rp[:, c * PSH : (c + 1) * PSH])}
        for c in range(NCORES)
    ]
    res = None
    for attempt in range(3):
        try:
            res = run_bass_kernel_spmd(
                nc, in_maps, core_ids=list(range(NCORES)), trace=_trace
            )
            break
        except Exception:
            if attempt == 2:
                raise
            import time
            time.sleep(2.0)

    min_gt = np.full(G, np.inf)
    min_pred = np.empty(P)
    for c, out in enumerate(res.results):
        gm = out["gmin"].astype(np.float64)          # [128 lane, GB block]
        min_gt = np.minimum(min_gt, gm.T.reshape(G) / SCALE)  # g = gb*128 + lane
        min_pred[c * PSH : (c + 1) * PSH] = (
            out["colacc"].astype(np.float64).min(axis=0) / SCALE
        )

    min_pred = np.maximum(min_pred, 0.0)
    min_gt = np.maximum(min_gt, 0.0)

    wp = np.asarray(w_pred, np.float64)
    wg = np.asarray(w_gt, np.float64)
    weighted_pred = (wp * min_pred).sum() / max(wp.sum(), EPS)
    weighted_gt = (wg * min_gt).sum() / max(wg.sum(), EPS)
    out = PRED_WEIGHT * weighted_pred + GT_WEIGHT * weighted_gt
    if _trace:
        return np.float32(out), res
    return np.float32(out)


# revision 9
# speedup vs baseline: 1.0159x; 1.0156x over previous
"""Density-aware Chamfer distance on 8 Trainium2 NeuronCores.

Problem: pred_points [16384,3], gt_points [16384,3], w_pred/w_gt [16384].
  d2[p,g] = max(|p|^2 + |g|^2 - 2 p.g, 0)
  out = sum(w_pred*min_g d2)/sum(w_pred) + sum(w_gt*min_p d2)/sum(w_gt)

Sharding: pred rows are split across the 8 cores (2048 each). Each core
computes its 2048 x 16384 distance tile entirely on-chip:

 - The d2 matrix block is produced on the TensorEngine as a K=30 bf16
   matmul: d2 = sum_k A[k,g] * B[k,p] with A = [g2, 1, gx, gy, gz] and
   B = [1, p2, -2px, -2py, -2pz], where every product is expanded into
   6 bf16-pair partial products (3-way bf16 split of each fp32 value),
   giving fp32-grade accuracy at full bf16 PE speed (K stays under 128
   so the extra rows are free).
 - Orientation: gt on partitions (128 gt-blocks), pred on the free dim
   (2048). Per block, PSUM holds 1024*d2 [128gt, 2048pred] in fp32 (the
   2^10 scale keeps nearest-neighbour distances in fp16 normal range).
 - ScalarE copies PSUM -> SBUF fp16. VectorE then (a) min-accumulates
   block pairs into a running colacc [128, 2048] (fp16 tensor_tensor at
   2x rate) for the min over gt, and (b) does a pairwise-min tree over
   the free dim (fp16 2x) + an 8-block-grouped reduce for the min over
   pred, which yields each gt-block's min_gt entries (one per lane).
 - Host combines: min_gt = elementwise min over the 8 cores' [128,128]
   block-min outputs; min_pred shard = column-min over the [128,2048]
   colacc; un-scale, clamp at 0 (max(.,0) commutes with min) and the
   weighted means are computed on host in float64.

The max(..., 0) clamp is applied after the min reductions (max(.,0) is
monotone, so it commutes with min).
"""

import numpy as np
import ml_dtypes

import concourse.bacc as bacc
import concourse.tile as tile
import concourse.mybir as mybir
from concourse.bass_utils import run_bass_kernel_spmd

F32 = mybir.dt.float32
F16 = mybir.dt.float16
BF16 = mybir.dt.bfloat16

P = 16384          # pred points
G = 16384          # gt points
NCORES = 8
PSH = P // NCORES  # 2048 pred per core
GB = G // 128      # 128 gt blocks per core
NCH = PSH // 512   # 4 matmul column chunks per block
K = 30             # 5 terms x 6 bf16-pair partial products

PRED_WEIGHT = 1.0
GT_WEIGHT = 1.0
EPS = 1e-9

# bf16-pair partial products kept from (x1+x2+x3)*(y1+y2+y3); dropped
# terms are O(2^-32) relative.
PAIRS = [(0, 0), (0, 1), (1, 0), (1, 1), (0, 2), (2, 0)]

# The on-device min pipeline runs in fp16; d2 is scaled by 2^10 (folded
# into the gt-side matmul rows) so typical nearest-neighbour distances
# (~1e-5) land in fp16's normal range. Overflowed large distances become
# inf, which min() ignores.
SCALE = 1024.0

_CACHED = {}


def _split3(x):
    """3-way bf16 split of a float64 array: x ~= s[0]+s[1]+s[2]."""
    out = []
    r = x
    for _ in range(3):
        h = r.astype(ml_dtypes.bfloat16).astype(np.float64)
        out.append(h)
        r = r - h
    return out


def _expand_rows(A, B):
    """A [5, n], B [5, m] float64 -> (L [30, n], R [30, m]) bf16 with
    sum_k L[k,i]*R[k,j] ~= sum_t A[t,i]*B[t,j]."""
    SA = [_split3(A[t]) for t in range(A.shape[0])]
    SB = [_split3(B[t]) for t in range(B.shape[0])]
    L, R = [], []
    for t in range(A.shape[0]):
        for (i, j) in PAIRS:
            L.append(SA[t][i])
            R.append(SB[t][j])
    return (np.stack(L).astype(ml_dtypes.bfloat16),
            np.stack(R).astype(ml_dtypes.bfloat16))


def _build_device_kernel():
    nc = bacc.Bacc("TRN2", target_bir_lowering=False)
    lg_d = nc.dram_tensor("lg", [K, G], BF16, kind="ExternalInput")
    rp_d = nc.dram_tensor("rp", [K, PSH], BF16, kind="ExternalInput")
    gmin_d = nc.dram_tensor("gmin", [128, GB], F32, kind="ExternalOutput")
    colacc_d = nc.dram_tensor("colacc", [128, PSH], F16, kind="ExternalOutput")

    with tile.TileContext(nc) as tc:
        with (
            tc.tile_pool(name="inp", bufs=1) as inp,
            tc.tile_pool(name="cpp", bufs=4) as cpp,
            tc.tile_pool(name="trp", bufs=3) as trp,
            tc.tile_pool(name="t3p", bufs=2) as t3p,
            tc.tile_pool(name="outp", bufs=1) as outp,
            tc.tile_pool(name="ps", bufs=2, space="PSUM") as ps,
        ):
            lg = inp.tile([K, G], BF16)
            rp = inp.tile([K, PSH], BF16)
            # chunked prefetch so block 0's matmuls start early
            for ch in range(8):
                nc.sync.dma_start(
                    lg[:, ch * (G // 8) : (ch + 1) * (G // 8)],
                    lg_d[:, ch * (G // 8) : (ch + 1) * (G // 8)],
                )
            nc.sync.dma_start(rp[:], rp_d[:])

            colacc = outp.tile([128, PSH], F16)
            nc.vector.memset(colacc[:], 60000.0)
            gmin = outp.tile([128, GB], F32)

            MIN = mybir.AluOpType.min
            # process gt blocks four at a time to amortize DVE op overheads
            for sg in range(GB // 4):
                cp = cpp.tile([128, 4, PSH], F16, tag="cp")
                for b in range(4):
                    gb = 4 * sg + b
                    acc = ps.tile([128, PSH], F32, tag="acc")
                    w = lg[:, 128 * gb : 128 * (gb + 1)]
                    for c in range(NCH):
                        nc.tensor.matmul(
                            acc[:, 512 * c : 512 * (c + 1)],
                            w,
                            rp[:, 512 * c : 512 * (c + 1)],
                            start=True,
                            stop=True,
                        )
                    nc.scalar.copy(cp[:, b, :], acc[:])

                # min over the 4 blocks (min_pred side): pair-min, fold, then
                # accumulate into colacc
                uu = trp.tile([128, 2, PSH], F16, tag="uu")
                nc.vector.tensor_tensor(
                    out=uu[:], in0=cp[:, 0::2, :], in1=cp[:, 1::2, :], op=MIN
                )
                v = trp.tile([128, PSH], F16, tag="v")
                nc.vector.tensor_tensor(
                    out=v[:], in0=uu[:, 0, :], in1=uu[:, 1, :], op=MIN
                )
                nc.vector.tensor_tensor(
                    out=colacc[:], in0=colacc[:], in1=v[:], op=MIN
                )

                # per-block pairwise-min tree over pred (min_gt side), all
                # four blocks folded per instruction
                t1 = trp.tile([128, 4, PSH // 2], F16, tag="t1")
                nc.vector.tensor_tensor(
                    out=t1[:],
                    in0=cp[:, :, : PSH // 2], in1=cp[:, :, PSH // 2 :],
                    op=MIN,
                )
                t2 = trp.tile([128, 4, PSH // 4], F16, tag="t2")
                nc.vector.tensor_tensor(
                    out=t2[:],
                    in0=t1[:, :, : PSH // 4], in1=t1[:, :, PSH // 4 :],
                    op=MIN,
                )
                # t3 goes into the 8-block gather buffer
                j = sg % 2
                if j == 0:
                    t3g = t3p.tile([128, 8, PSH // 8], F16, tag="t3g")
                nc.vector.tensor_tensor(
                    out=t3g[:, 4 * j : 4 * j + 4, :],
                    in0=t2[:, :, : PSH // 8], in1=t2[:, :, PSH // 8 :],
                    op=MIN,
                )
                if j == 1:
                    gb0 = 4 * (sg - 1)
                    nc.vector.tensor_reduce(
                        gmin[:, gb0 : gb0 + 8], t3g[:],
                        axis=mybir.AxisListType.X, op=MIN,
                    )

            nc.sync.dma_start(gmin_d[:], gmin[:])
            nc.sync.dma_start(colacc_d[:], colacc[:])

    nc.compile()
    return nc


def _get_nc():
    if "nc" not in _CACHED:
        _CACHED["nc"] = _build_device_kernel()
    return _CACHED["nc"]


def kernel(pred_points, gt_points, w_pred, w_gt, _trace=False):
    pred = np.asarray(pred_points, np.float64)
    gt = np.asarray(gt_points, np.float64)
    p2 = (pred * pred).sum(1)
    g2 = (gt * gt).sum(1)

    A = SCALE * np.stack([g2, np.ones(G), gt[:, 0], gt[:, 1], gt[:, 2]])  # [5, G]
    B = np.stack([np.ones(P), p2, -2 * pred[:, 0], -2 * pred[:, 1],
                  -2 * pred[:, 2]])                                     # [5, P]
    Lg, Rp = _expand_rows(A, B)  # [30, G], [30, P] bf16

    nc = _get_nc()
    in_maps = [
        {"lg": Lg, "rp": np.ascontiguousarray(Rp[:, c * PSH : (c + 1) * PSH])}
        for c in range(NCORES)
    ]
    res = None
    for attempt in range(3):
        try:
            res = run_bass_kernel_spmd(
                nc, in_maps, core_ids=list(range(NCORES)), trace=_trace
            )
            break
        except Exception:
            if attempt == 2:
                raise
            import time
            time.sleep(2.0)

    min_gt = np.full(G, np.inf)
    min_pred = np.empty(P)
    for c, out in enumerate(res.results):
        gm = out["gmin"].astype(np.float64)          # [128 lane, GB block]
        min_gt = np.minimum(min_gt, gm.T.reshape(G) / SCALE)  # g = gb*128 + lane
        min_pred[c * PSH : (c + 1) * PSH] = (
            out["colacc"].astype(np.float64).min(axis=0) / SCALE
        )

    min_pred = np.maximum(min_pred, 0.0)
    min_gt = np.maximum(min_gt, 0.0)

    wp = np.asarray(w_pred, np.float64)
    wg = np.asarray(w_gt, np.float64)
    weighted_pred = (wp * min_pred).sum() / max(wp.sum(), EPS)
    weighted_gt = (wg * min_gt).sum() / max(wg.sum(), EPS)
    out = PRED_WEIGHT * weighted_pred + GT_WEIGHT * weighted_gt
    if _trace:
        return np.array(out, dtype=np.float32), res
    return np.array(out, dtype=np.float32)
